# revision 10
# baseline (speedup 1.0000x reference)
"""Embedding lookup (KVEmbedding) on 8 TRN2 NeuronCores.

Strategy: the 256 MB table fits in HBM, so replicate it on every core and
shard the indices along batch (2048 rows/core). Each core runs a 3-stage
DMA pipeline over 25 tiles of 16384 lookups:
  1. HWDGE load of a [128, K] int32 index tile (contiguous, 64 KB)
  2. SWDGE indirect gather table[idx] -> SBUF [128, K*64] (16384 rows/instr)
  3. HWDGE store of the [128, K*64] f32 tile to the output (contiguous 4 MB)
No collectives needed; output shards concatenate on host.
"""

import numpy as np

BATCH, HIST = 16384, 200
VOCAB, D = 1_000_000, 64
NCORES = 8
ROWS_PER_CORE = BATCH // NCORES          # 2048
FLAT = ROWS_PER_CORE * HIST              # 409600 lookups per core
P = 128                                  # SBUF partitions
K = 128                                  # indices per partition per tile
TILE_ROWS = P * K                        # 16384
NTILES = FLAT // TILE_ROWS               # 25

_built = None


def _build(flat=FLAT, vocab=VOCAB, d=D, k=K, bufs=3):
    """Raw-Bass 2-queue pipeline.

    Tile's auto-semaphores emit 2 embedded waits on steady-state gathers
    (WAW on the slot's previous gather + WAR on the freeing store), but the
    DMA ISA struct holds only one sync-wait -> codegen ICE. Raw Bass keeps
    every DMA at zero embedded waits (standalone sequencer waits) and one
    sem update.
      gpsimd (Pool/SWDGE):  indirect gathers  table[idx] -> SBUF slot i%bufs
      sync   (SP/HWDGE):    idx preload, then contiguous stores slot -> out
    """
    from contextlib import ExitStack

    import concourse.bass as bass
    import concourse.mybir as mybir

    ntiles = flat // (P * k)
    assert ntiles * P * k == flat

    nc = bass.Bass()
    idx = nc.declare_dram_parameter("idx", [flat], mybir.dt.int32, isOutput=False)
    table = nc.declare_dram_parameter(
        "table", [vocab, d], mybir.dt.float32, isOutput=False
    )
    out = nc.declare_dram_parameter(
        "out", [flat, d], mybir.dt.float32, isOutput=True
    )

    idx_t = idx[:].rearrange("(n p k) -> p n k", p=P, k=k)        # [128, n, k]
    out_t = out[:].rearrange("(n p k) d -> n p (k d)", p=P, k=k)  # [n, 128, k*d]

    # One gather-sem and one store-sem PER SLOT: a shared counter would let
    # partial +1 increments from a later in-flight DMA satisfy an earlier
    # instruction's 16*(i+1) threshold (DMA completions interleave across
    # the 16 engines). Per-slot, at most one incrementer is in flight, so
    # every wait value is exact.
    with ExitStack() as ctx:
        it = ctx.enter_context(nc.sbuf_tensor([P, ntiles * k], mybir.dt.int32))
        ot = ctx.enter_context(
            nc.sbuf_tensor([P, bufs * k * d], mybir.dt.float32)
        )
        ls = ctx.enter_context(nc.semaphore("ls"))
        gsem = [ctx.enter_context(nc.semaphore(f"gs{s}")) for s in range(bufs)]
        ssem = [ctx.enter_context(nc.semaphore(f"ss{s}")) for s in range(bufs)]
        block = ctx.enter_context(nc.Block())

        @block.sync
        def _(sync):
            sync.dma_start(
                out=it[:].rearrange("p (n k) -> p n k", k=k), in_=idx_t
            ).then_inc(ls, 16)
            for i in range(ntiles):
                s, c = i % bufs, i // bufs
                # all k gathers of this group must have completed
                sync.wait_ge(gsem[s], 16 * k * (c + 1))
                sync.dma_start(
                    out=out_t[i], in_=ot[:, s * k * d : (s + 1) * k * d]
                ).then_inc(ssem[s], 16)

        @block.gpsimd
        def _(gpsimd):
            # HW descriptor generation consumes ONE index per partition per
            # indirect DMA (multi-index offset APs gather garbage beyond
            # col 0), so each group is k instructions of 128 rows each.
            gpsimd.wait_ge(ls, 16)
            for i in range(ntiles):
                s, c = i % bufs, i // bufs
                if c >= 1:
                    gpsimd.wait_ge(ssem[s], 16 * c)
                for j in range(k):
                    gpsimd.indirect_dma_start(
                        out=ot[:, (s * k + j) * d : (s * k + j + 1) * d],
                        out_offset=None,
                        in_=table[:, :],
                        in_offset=bass.IndirectOffsetOnAxis(
                            ap=it[:, i * k + j : i * k + j + 1], axis=0
                        ),
                    ).then_inc(gsem[s], 16)

    return nc


def run(indices, table, dummy=None, trace=False):
    global _built
    from concourse.bass_utils import run_bass_kernel_spmd

    if _built is None:
        _built = _build()
    nc = _built

    idx32 = np.ascontiguousarray(
        np.asarray(indices).reshape(NCORES, FLAT).astype(np.int32)
    )
    tab = np.ascontiguousarray(np.asarray(table), dtype=np.float32)
    in_maps = [{"idx": idx32[c], "table": tab} for c in range(NCORES)]
    kres = run_bass_kernel_spmd(nc, in_maps, list(range(NCORES)), trace=trace)
    out = np.concatenate(
        [kres.results[c]["out"].reshape(ROWS_PER_CORE, HIST, D) for c in range(NCORES)],
        axis=0,
    )
    return out, kres


def kernel(indices, table, dummy=None):
    return run(indices, table, dummy)[0]
